# revision 1
# baseline (speedup 1.0000x reference)
"""Trainium2 Bass kernel for batched weighted complex Gram matrices.

Reference computation (per batch b):
    out_r = R^T diag(w) R + I^T diag(w) I      (symmetric)
    out_i = I^T diag(w) R - R^T diag(w) I      (antisymmetric)
with R = input_real[b] (S=1024, D=256), I = input_imag[b], w = weights[b].

Sharding: data-parallel over batch, 4 batches per NeuronCore x 8 cores.

Per-core scheme (all compute on-chip, fp32r matmuls on the PE):
    x_sb = [I | R]            fp32   (one strided DMA per tensor per batch)
    xr   = [I | R | -I]       f32r   (ACT rounding copy + DVE negate)
    wr   = w*R, wi = w*I      f32r   (DVE tensor_scalar, per-partition w)
    psum_a += WI_a^T [I|R]    -> [ out_r | G3 ]     (N=512 moving window)
    psum_a += WR_a^T [R|-I]   -> [ out_r | -G4 ]    (overlapping window)
    => psum_a = [out_r_a | out_i_a]; copy PSUM->SBUF (ACT/DVE), DMA out.
"""

import sys

if "/opt/trn_rl_repo" not in sys.path:
    sys.path.insert(0, "/opt/trn_rl_repo")

import numpy as np

B, S, D = 32, 1024, 256
NCORES = 8
NB = B // NCORES          # batches per core
NCH = S // 128            # contraction chunks per batch

_compiled = {}


def _build():
    import concourse.bacc as bacc
    import concourse.tile as tile
    import concourse.mybir as mybir

    f32 = mybir.dt.float32
    f32r = mybir.dt.float32r

    nc = bacc.Bacc("TRN2", target_bir_lowering=False, debug=False)
    r_d = nc.dram_tensor("r", [NB, S, D], f32, kind="ExternalInput")
    i_d = nc.dram_tensor("i", [NB, S, D], f32, kind="ExternalInput")
    w_d = nc.dram_tensor("w", [NB, S], f32, kind="ExternalInput")
    or_d = nc.dram_tensor("o_r", [NB, D, D], f32, kind="ExternalOutput")
    oi_d = nc.dram_tensor("o_i", [NB, D, D], f32, kind="ExternalOutput")

    with tile.TileContext(nc) as tc:
        with (
            tc.tile_pool(name="wpool", bufs=1) as wpool,
            tc.tile_pool(name="xp", bufs=3) as xp,
            tc.tile_pool(name="mp", bufs=2) as mp,
            tc.tile_pool(name="op", bufs=2) as op,
            tc.tile_pool(name="ps", bufs=2, space="PSUM") as ps,
        ):
            # per-partition weight scalars: w_sc[p, b, c] = w[b, c*128+p]
            w_sc = wpool.tile([128, NB, NCH], f32)
            nc.sync.dma_start(w_sc[:], w_d.rearrange("b (c p) -> p b c", p=128))

            for b in range(NB):
                x_sb = xp.tile([128, NCH, 512], f32, name="x_sb")
                nc.sync.dma_start(
                    x_sb[:, :, 0:256], i_d[b].rearrange("(c p) d -> p c d", p=128)
                )
                nc.sync.dma_start(
                    x_sb[:, :, 256:512], r_d[b].rearrange("(c p) d -> p c d", p=128)
                )

                xr = mp.tile([128, NCH, 768], f32r, name="xr")
                wr = mp.tile([128, NCH, 256], f32r, name="wr")
                wi = mp.tile([128, NCH, 256], f32r, name="wi")
                for c in range(NCH):
                    # rounded moving operand [I | R | -I]
                    nc.scalar.copy(xr[:, c, 0:512], x_sb[:, c, :])
                    nc.vector.tensor_scalar_mul(
                        xr[:, c, 512:768], x_sb[:, c, 0:256], -1.0
                    )
                    # weighted stationary operands
                    nc.vector.tensor_scalar_mul(
                        wr[:, c, :], x_sb[:, c, 256:512], w_sc[:, b, c:c + 1]
                    )
                    nc.vector.tensor_scalar_mul(
                        wi[:, c, :], x_sb[:, c, 0:256], w_sc[:, b, c:c + 1]
                    )

                psum = [ps.tile([128, 512], f32, name=f"psum{a}") for a in range(2)]
                for c in range(NCH):
                    for a in range(2):
                        nc.tensor.matmul(
                            psum[a][:],
                            wi[:, c, 128 * a:128 * a + 128],
                            xr[:, c, 0:512],
                            start=(c == 0),
                            stop=False,
                            skip_group_check=True,
                        )
                        nc.tensor.matmul(
                            psum[a][:],
                            wr[:, c, 128 * a:128 * a + 128],
                            xr[:, c, 256:768],
                            start=False,
                            stop=(c == NCH - 1),
                            skip_group_check=True,
                        )

                out_sb = op.tile([128, 2, 512], f32, name="out_sb")
                for a in range(2):
                    nc.scalar.copy(out_sb[:, a, 0:256], psum[a][:, 0:256])
                    nc.vector.tensor_copy(out_sb[:, a, 256:512], psum[a][:, 256:512])
                    nc.sync.dma_start(
                        or_d[b, 128 * a:128 * a + 128, :], out_sb[:, a, 0:256]
                    )
                    nc.sync.dma_start(
                        oi_d[b, 128 * a:128 * a + 128, :], out_sb[:, a, 256:512]
                    )

    nc.compile()
    return nc


def _get_nc():
    if "nc" not in _compiled:
        _compiled["nc"] = _build()
    return _compiled["nc"]


def run(input_real, input_imag, weights, trace=False):
    from concourse.bass_utils import run_bass_kernel_spmd

    nc = _get_nc()
    in_maps = []
    for c in range(NCORES):
        sl = slice(NB * c, NB * (c + 1))
        in_maps.append(
            {
                "r": np.ascontiguousarray(input_real[sl], dtype=np.float32),
                "i": np.ascontiguousarray(input_imag[sl], dtype=np.float32),
                "w": np.ascontiguousarray(weights[sl], dtype=np.float32),
            }
        )
    res = run_bass_kernel_spmd(
        nc, in_maps, core_ids=list(range(NCORES)), trace=trace
    )
    out_r = np.concatenate([res.results[c]["o_r"] for c in range(NCORES)], axis=0)
    out_i = np.concatenate([res.results[c]["o_i"] for c in range(NCORES)], axis=0)
    return (out_r, out_i), res


def kernel(input_real, input_imag, weights):
    (out_r, out_i), _ = run(input_real, input_imag, weights, trace=False)
    return (out_r, out_i)


# revision 2
# speedup vs baseline: 1.0604x; 1.0604x over previous
"""Trainium2 Bass kernel for batched weighted complex Gram matrices.

Reference computation (per batch b):
    out_r = R^T diag(w) R + I^T diag(w) I      (symmetric)
    out_i = I^T diag(w) R - R^T diag(w) I      (antisymmetric)
with R = input_real[b] (S=1024, D=256), I = input_imag[b], w = weights[b].

Sharding: data-parallel over batch, 4 batches per NeuronCore x 8 cores.

Per-core scheme (all compute on-chip, fp32r matmuls on the PE):
    x_sb = [I | R]            fp32   (chunked strided DMAs)
    xr   = [I | R | -I]       f32r   (ACT rounding copy + DVE negate)
    wr   = w*R, wi = w*I      f32r   (DVE tensor_scalar, per-partition w)
    psum_a += WI_a^T [I|R]    -> [ out_r | G3 ]     (N=512 moving window)
    psum_a += WR_a^T [R|-I]   -> [ out_r | -G4 ]    (overlapping window)
    => psum_a = [out_r_a | out_i_a]; copy PSUM->SBUF (ACT/DVE), DMA out.
"""

import sys

if "/opt/trn_rl_repo" not in sys.path:
    sys.path.insert(0, "/opt/trn_rl_repo")

import numpy as np

B, S, D = 32, 1024, 256
NCORES = 8
NB = B // NCORES          # batches per core
NCH = S // 128            # contraction chunks per batch
DMA_SPLIT = 2             # chunks per input-DMA piece

_compiled = {}


def _build():
    import concourse.bacc as bacc
    import concourse.tile as tile
    import concourse.mybir as mybir

    f32 = mybir.dt.float32
    f32r = mybir.dt.float32r

    nc = bacc.Bacc("TRN2", target_bir_lowering=False, debug=False)
    r_d = nc.dram_tensor("r", [NB, S, D], f32, kind="ExternalInput")
    i_d = nc.dram_tensor("i", [NB, S, D], f32, kind="ExternalInput")
    # host-pretransposed weights: w_t[p, b*NCH+c] = weights[b, c*128+p]
    wt_d = nc.dram_tensor("w_t", [128, NB * NCH], f32, kind="ExternalInput")
    or_d = nc.dram_tensor("o_r", [NB, D, D], f32, kind="ExternalOutput")
    oi_d = nc.dram_tensor("o_i", [NB, D, D], f32, kind="ExternalOutput")

    with tile.TileContext(nc) as tc:
        with (
            tc.tile_pool(name="wpool", bufs=1) as wpool,
            tc.tile_pool(name="xp", bufs=3) as xp,
            tc.tile_pool(name="mp", bufs=2) as mp,
            tc.tile_pool(name="op", bufs=2) as op,
            tc.tile_pool(name="ps", bufs=3, space="PSUM") as ps,
        ):
            w_sc = wpool.tile([128, NB * NCH], f32)
            nc.sync.dma_start(w_sc[:], wt_d[:])

            ir_re = i_d.rearrange("b (c p) d -> b p c d", p=128)
            rr_re = r_d.rearrange("b (c p) d -> b p c d", p=128)

            for b in range(NB):
                x_sb = xp.tile([128, NCH, 512], f32, name="x_sb")
                xr = mp.tile([128, NCH, 768], f32r, name="xr")
                wr = mp.tile([128, NCH, 256], f32r, name="wr")
                wi = mp.tile([128, NCH, 256], f32r, name="wi")
                psum = [ps.tile([128, 512], f32, name=f"psum{a}") for a in range(2)]

                for c in range(NCH):
                    if c % DMA_SPLIT == 0:
                        ce = c + DMA_SPLIT
                        nc.sync.dma_start(
                            x_sb[:, c:ce, 0:256], ir_re[b, :, c:ce, :]
                        )
                        nc.sync.dma_start(
                            x_sb[:, c:ce, 256:512], rr_re[b, :, c:ce, :]
                        )
                    # rounded moving operand [I | R | -I]
                    nc.scalar.copy(xr[:, c, 0:512], x_sb[:, c, :])
                    nc.vector.tensor_scalar_mul(
                        xr[:, c, 512:768], x_sb[:, c, 0:256], -1.0
                    )
                    # weighted stationary operands
                    wcol = b * NCH + c
                    nc.vector.tensor_scalar_mul(
                        wr[:, c, :], x_sb[:, c, 256:512], w_sc[:, wcol:wcol + 1]
                    )
                    nc.vector.tensor_scalar_mul(
                        wi[:, c, :], x_sb[:, c, 0:256], w_sc[:, wcol:wcol + 1]
                    )
                    for a in range(2):
                        nc.tensor.matmul(
                            psum[a][:],
                            wi[:, c, 128 * a:128 * a + 128],
                            xr[:, c, 0:512],
                            start=(c == 0),
                            stop=False,
                            skip_group_check=True,
                        )
                        nc.tensor.matmul(
                            psum[a][:],
                            wr[:, c, 128 * a:128 * a + 128],
                            xr[:, c, 256:768],
                            start=False,
                            stop=(c == NCH - 1),
                            skip_group_check=True,
                        )

                out_sb = op.tile([128, 2, 512], f32, name="out_sb")
                for a in range(2):
                    nc.scalar.copy(out_sb[:, a, 0:256], psum[a][:, 0:256])
                    nc.vector.tensor_copy(out_sb[:, a, 256:512], psum[a][:, 256:512])
                    nc.sync.dma_start(
                        or_d[b, 128 * a:128 * a + 128, :], out_sb[:, a, 0:256]
                    )
                    nc.sync.dma_start(
                        oi_d[b, 128 * a:128 * a + 128, :], out_sb[:, a, 256:512]
                    )

    nc.compile()
    return nc


def _get_nc():
    if "nc" not in _compiled:
        _compiled["nc"] = _build()
    return _compiled["nc"]


def run(input_real, input_imag, weights, trace=False):
    from concourse.bass_utils import run_bass_kernel_spmd

    nc = _get_nc()
    w = np.asarray(weights, dtype=np.float32)
    in_maps = []
    for c in range(NCORES):
        sl = slice(NB * c, NB * (c + 1))
        # w_t[p, b*NCH+ch] = w[b, ch*128+p]
        w_t = np.ascontiguousarray(
            w[sl].reshape(NB, NCH, 128).transpose(2, 0, 1).reshape(128, NB * NCH)
        )
        in_maps.append(
            {
                "r": np.ascontiguousarray(input_real[sl], dtype=np.float32),
                "i": np.ascontiguousarray(input_imag[sl], dtype=np.float32),
                "w_t": w_t,
            }
        )
    res = run_bass_kernel_spmd(
        nc, in_maps, core_ids=list(range(NCORES)), trace=trace
    )
    out_r = np.concatenate([res.results[c]["o_r"] for c in range(NCORES)], axis=0)
    out_i = np.concatenate([res.results[c]["o_i"] for c in range(NCORES)], axis=0)
    return (out_r, out_i), res


def kernel(input_real, input_imag, weights):
    (out_r, out_i), _ = run(input_real, input_imag, weights, trace=False)
    return (out_r, out_i)
